# revision 7
# baseline (speedup 1.0000x reference)
"""Trainium2 Bass kernel for nn_MultiHeadAttention_54305566490675.

Sharding: 8 cores; core i handles batch b = i//2 and head group hg = i%2
(4 of the 8 heads).  All score-path work is per (b, h) and fully local.
The out-projection is computed per-core over its own head columns
(partial sums); the host adds the two partials per batch (the unshard
reduction).  Everything heavy runs on device.

Math per (b, h), matching reference.py:
  qh/kh = (x @ Wq[own].T + bq) rope'd (interleaved pairs, base 10000)
  scores = qh @ kh.T / sqrt(64)           (1/8 folded into q's cos/sin)
  masked = scores + B2                    (B2 = 0 valid, -1e38 invalid)
  p  = exp(masked)        (no max-sub needed: |scores| is small)
  cum = inclusive-scan(p);  tot = sum(p);  rs = 1/tot
  w = (cum - tot) * pdist * (-g^2 * rs)   >= 0
  te = exp(-sqrt(w)) = exp(g * dist),   clipped below at 1e-5
  s2 = masked*te_clip + alibi;  attn = softmax(s2)  (max-subtracted)
  ctx = attn @ vh;   pout = ctx_heads @ Wo[:, own].T + bo(even core)
"""

import math
import sys

import numpy as np

sys.path.insert(0, "/opt/trn_rl_repo")

import concourse.bass as bass
import concourse.bacc as bacc
import concourse.tile as tile
from concourse import mybir
from concourse.bass_utils import run_bass_kernel_spmd
from concourse.masks import make_identity

B, S, D, H = 4, 1024, 512, 8
DK = D // H          # 64
HG = H // 2          # 4 heads per core
NCORES = 8
SC = S // 128        # 8 seq chunks
DC = D // 128        # 4 d chunks

F32 = mybir.dt.float32
BF16 = mybir.dt.bfloat16
FP16 = mybir.dt.float16

Alu = mybir.AluOpType
Act = mybir.ActivationFunctionType

NEG_BIG = -1.0e38
CLIP_LO = 1.0e-5


def _build_program():
    nc = bacc.Bacc(
        "TRN2", target_bir_lowering=False, debug=False, num_devices=NCORES
    )

    def din(name, shape, dtype=F32):
        return nc.dram_tensor(name, shape, dtype, kind="ExternalInput").ap()

    def dout(name, shape, dtype=F32):
        return nc.dram_tensor(name, shape, dtype, kind="ExternalOutput").ap()

    ins = dict(
        qT=din("qT", [DC, 128, S]),        # q[b].T, d-chunked
        kT=din("kT", [DC, 128, S]),
        vT=din("vT", [DC, 128, S]),
        wqT=din("wqT", [DC, 128, 256]),    # Wq[own,:].T  (also used for k)
        wvT=din("wvT", [DC, 128, 256]),
        woT=din("woT", [2, 128, 512]),     # Wo[:, own].T in 2 row-chunks
        bq=din("bq", [1, 256]),
        bv=din("bv", [1, 256]),
        bo=din("bo", [1, 512]),            # bo on even cores, zeros on odd
        cosq=din("cosq", [128, S]),        # [2-head-stacked dk, s], /8 folded
        sinq=din("sinq", [128, S]),
        cosk=din("cosk", [128, S]),
        sink=din("sink", [128, S]),
        rmat=din("rmat", [128, 128]),      # rot lhsT:  rot = R @ xT
        b2=din("b2", [SC, 128, S], BF16),  # additive mask, i-chunked
        pdist=din("pdist", [SC, 128, S], FP16),  # |i - j|, i-chunked
        alibi=din("alibi", [HG, S], FP16),  # slopes[own_h] * j
        g2=din("g2", [1, HG]),             # -softplus(gamma_own)^2
    )
    outs = dict(
        attn_out=dout("attn_out", [HG, S, S]),
        pout=dout("pout", [S, D]),
    )

    from contextlib import ExitStack
    with tile.TileContext(nc) as tc, ExitStack() as stk:
        _body(tc, stk, ins, outs)

    nc.compile()
    return nc


def _body(tc, stk, ins, outs):
    nc = tc.nc
    attn_out = outs["attn_out"]
    pout = outs["pout"]

    def pool(**kw):
        return stk.enter_context(tc.tile_pool(**kw))

    const = pool(name="const", bufs=1)
    persist = pool(name="persist", bufs=1)
    xts = pool(name="xts", bufs=2)
    ropetmp = pool(name="ropetmp", bufs=3)
    work = pool(name="work", bufs=2)
    attnp = pool(name="attnp", bufs=2)
    attntp = pool(name="attntp", bufs=2)
    scal = pool(name="scal", bufs=3)
    pop = pool(name="pop", bufs=2)
    # 8 PSUM banks total: psA = 2x[128,1024] (4 banks), psB = 4x[128,512]
    psA = pool(name="psA", bufs=2, space="PSUM")
    psB = pool(name="psB", bufs=4, space="PSUM")

    # ---- constants in SBUF ----
    ident = const.tile([128, 128], F32)
    make_identity(nc, ident)
    identb = const.tile([128, 128], BF16)
    nc.vector.tensor_copy(out=identb, in_=ident)
    ones_row = const.tile([1, 512], F32)
    nc.vector.memset(ones_row, 1.0)

    rmat_sb = const.tile([128, 128], F32)
    nc.sync.dma_start(out=rmat_sb, in_=ins["rmat"])
    cs = {}
    for name in ("cosq", "sinq", "cosk", "sink"):
        t = const.tile([128, S], F32, name=name + "_sb")
        nc.sync.dma_start(out=t, in_=ins[name])
        cs[name] = t
    wq_sb = const.tile([128, DC, 256], F32)
    wv_sb = const.tile([128, DC, 256], F32)
    for dc in range(DC):
        nc.sync.dma_start(out=wq_sb[:, dc, :], in_=ins["wqT"][dc])
        nc.sync.dma_start(out=wv_sb[:, dc, :], in_=ins["wvT"][dc])
    wo_sb = const.tile([128, 2, 512], F32)
    for hp in range(2):
        nc.sync.dma_start(out=wo_sb[:, hp, :], in_=ins["woT"][hp])
    bq_sb = const.tile([1, 256], F32)
    nc.sync.dma_start(out=bq_sb, in_=ins["bq"])
    bv_sb = const.tile([1, 256], F32)
    nc.sync.dma_start(out=bv_sb, in_=ins["bv"])
    bo_sb = const.tile([1, 512], F32)
    nc.sync.dma_start(out=bo_sb, in_=ins["bo"])

    b2_sb = const.tile([128, SC, S], BF16)
    pd_sb = const.tile([128, SC, S], FP16)
    for ic in range(SC):
        nc.sync.dma_start(out=b2_sb[:, ic, :], in_=ins["b2"][ic])
        nc.sync.dma_start(out=pd_sb[:, ic, :], in_=ins["pdist"][ic])
    al = ins["alibi"]
    al_sb = const.tile([128, HG, S], FP16)
    nc.sync.dma_start(
        out=al_sb,
        in_=bass.AP(tensor=al.tensor, offset=al.offset,
                    ap=[[0, 128], [S, HG], [1, S]]),
    )
    g2ap = ins["g2"]
    g2_sb = const.tile([128, HG], F32)
    nc.sync.dma_start(
        out=g2_sb,
        in_=bass.AP(tensor=g2ap.tensor, offset=g2ap.offset,
                    ap=[[0, 128], [1, HG]]),
    )

    # ---- persistent activations ----
    qhr = [persist.tile([128, S], F32, name=f"qhr{hp}") for hp in range(2)]
    khr = [persist.tile([128, S], F32, name=f"khr{hp}") for hp in range(2)]
    vh_sb = persist.tile([128, SC, 256], F32)
    ctx_sb = [persist.tile([128, S], F32, name=f"ctx{hp}") for hp in range(2)]

    # ---- phase A: q/k projections + rope (transposed orientation) ----
    for hp in range(2):
        hpsl = slice(hp * 128, (hp + 1) * 128)
        ps = {}
        for t in ("q", "k"):
            for sh in range(2):
                ps[(t, sh)] = psB.tile([128, 512], F32,
                                       name=f"ps_{t}{sh}", tag="ps512")
        for dc in range(DC):
            qt = xts.tile([128, S], F32, name="qt", tag="xt")
            nc.sync.dma_start(out=qt, in_=ins["qT"][dc])
            kt = xts.tile([128, S], F32, name="kt", tag="xt")
            nc.sync.dma_start(out=kt, in_=ins["kT"][dc])
            for sh in range(2):
                ssl = slice(sh * 512, (sh + 1) * 512)
                nc.tensor.matmul(ps[("q", sh)], lhsT=wq_sb[:, dc, hpsl],
                                 rhs=qt[:, ssl], start=(dc == 0), stop=False)
                nc.tensor.matmul(ps[("k", sh)], lhsT=wq_sb[:, dc, hpsl],
                                 rhs=kt[:, ssl], start=(dc == 0), stop=False)
        for t in ("q", "k"):
            for sh in range(2):
                nc.tensor.matmul(ps[(t, sh)], lhsT=bq_sb[0:1, hpsl],
                                 rhs=ones_row[0:1, :], start=False, stop=True)
        for t in ("q", "k"):
            cost = cs["cosq" if t == "q" else "cosk"]
            sint = cs["sinq" if t == "q" else "sink"]
            dst = qhr[hp] if t == "q" else khr[hp]
            for sh in range(2):
                ssl = slice(sh * 512, (sh + 1) * 512)
                # emit both readers of the proj psum first so its slot
                # frees before the rot matmul needs one
                raw = ropetmp.tile([128, 512], F32, name="raw", tag="raw")
                nc.scalar.copy(out=raw, in_=ps[(t, sh)])
                t2 = ropetmp.tile([128, 512], F32, name="t2", tag="t2")
                nc.vector.tensor_tensor(out=t2, in0=ps[(t, sh)],
                                        in1=cost[:, ssl], op=Alu.mult)
                rotp = psB.tile([128, 512], F32, name="rotp", tag="ps512")
                nc.tensor.matmul(rotp, lhsT=rmat_sb, rhs=raw,
                                 start=True, stop=True)
                t1 = ropetmp.tile([128, 512], F32, name="t1", tag="t1")
                nc.vector.tensor_tensor(out=t1, in0=rotp, in1=sint[:, ssl],
                                        op=Alu.mult)
                nc.gpsimd.tensor_tensor(out=dst[:, ssl], in0=t2, in1=t1,
                                        op=Alu.add)

    # ---- vh projection (normal orientation [s, 4*64]) ----
    vts = []
    for dc in range(DC):
        vt = xts.tile([128, S], F32, name=f"vt{dc}", tag=f"vt{dc}", bufs=1)
        nc.sync.dma_start(out=vt, in_=ins["vT"][dc])
        vts.append(vt)
    for sc in range(SC):
        pv = psB.tile([128, 256], F32, name="pv", tag="ps512")
        for dc in range(DC):
            nc.tensor.matmul(pv, lhsT=vts[dc][:, sc * 128:(sc + 1) * 128],
                             rhs=wv_sb[:, dc, :], start=(dc == 0), stop=False)
        nc.tensor.matmul(pv, lhsT=ones_row[0:1, :128], rhs=bv_sb,
                         start=False, stop=True)
        nc.scalar.copy(out=vh_sb[:, sc, :], in_=pv)

    # ---- phase B: per (head, i-chunk) score pipeline ----
    for hp in range(2):
        for l in range(2):
            lg = 2 * hp + l          # local head index 0..3
            hsl = slice(64 * l, 64 * l + 64)
            for ic in range(SC):
                isl = slice(ic * 128, (ic + 1) * 128)
                pscore = psA.tile([128, S], F32, name="pscore", tag="ps1024")
                for jh in range(2):
                    jsl = slice(jh * 512, (jh + 1) * 512)
                    nc.tensor.matmul(pscore[:, jsl], lhsT=qhr[hp][hsl, isl],
                                     rhs=khr[hp][hsl, jsl],
                                     start=True, stop=False)
                    nc.tensor.matmul(pscore[:, jsl], lhsT=identb,
                                     rhs=b2_sb[:, ic, jsl],
                                     start=False, stop=True)

                p = work.tile([128, S], F32, name="p", tag="wa")
                totp = scal.tile([128, 1], F32, name="totp", tag="totp")
                nc.scalar.activation(out=p, in_=pscore, func=Act.Exp,
                                     accum_out=totp)
                cum = work.tile([128, S], F32, name="cum", tag="wb")
                nc.vector.tensor_tensor_scan(
                    out=cum, data0=p, data1=p, initial=0.0,
                    op0=Alu.add, op1=Alu.bypass)
                rs = scal.tile([128, 1], F32, name="rs", tag="rs")
                nc.vector.reciprocal(out=rs, in_=totp)
                ng2rs = scal.tile([128, 1], F32, name="ng2rs", tag="ng2rs")
                nc.vector.tensor_scalar_mul(ng2rs, rs, g2_sb[:, lg:lg + 1])
                prod = work.tile([128, S], F32, name="prod", tag="wc")
                nc.vector.scalar_tensor_tensor(
                    out=prod, in0=cum, scalar=totp, in1=pd_sb[:, ic, :],
                    op0=Alu.subtract, op1=Alu.mult)
                w = work.tile([128, S], F32, name="w", tag="wa")
                nc.vector.tensor_scalar(
                    out=w, in0=prod, scalar1=ng2rs, scalar2=0.0,
                    op0=Alu.mult, op1=Alu.max)
                y = work.tile([128, S], F32, name="y", tag="wb")
                nc.scalar.activation(out=y, in_=w, func=Act.Sqrt)
                te = work.tile([128, S], F32, name="te", tag="wc")
                nc.scalar.activation(out=te, in_=y, func=Act.Exp, scale=-1.0)
                s2m = work.tile([128, S], F32, name="s2m", tag="wa")
                nc.vector.scalar_tensor_tensor(
                    out=s2m, in0=te, scalar=CLIP_LO, in1=pscore,
                    op0=Alu.max, op1=Alu.mult)
                m2 = work.tile([128, S], F32, name="m2", tag="wb")
                nc.gpsimd.tensor_tensor(out=m2, in0=s2m,
                                        in1=al_sb[:, lg, :], op=Alu.add)
                mx2 = scal.tile([128, 1], F32, name="mx2", tag="mx2")
                nc.vector.tensor_reduce(out=mx2, in_=m2,
                                        axis=mybir.AxisListType.X, op=Alu.max)
                nmx2 = scal.tile([128, 1], F32, name="nmx2", tag="nmx2")
                nc.vector.tensor_scalar_mul(nmx2, mx2, -1.0)
                p2 = work.tile([128, S], F32, name="p2", tag="wc")
                sum2 = scal.tile([128, 1], F32, name="sum2", tag="sum2")
                nc.scalar.activation(out=p2, in_=m2, func=Act.Exp,
                                     bias=nmx2, accum_out=sum2)
                rs2 = scal.tile([128, 1], F32, name="rs2", tag="rs2")
                nc.vector.reciprocal(out=rs2, in_=sum2)
                attn = attnp.tile([128, S], F32, name="attn", tag="attn")
                nc.vector.tensor_scalar_mul(attn, p2, rs2)
                nc.sync.dma_start(out=attn_out[lg, isl, :], in_=attn)

                # ctx.T[dk, i] += vh.T @ attn.T
                attnt = attntp.tile([128, SC, 128], F32, name="attnt",
                                    tag="attnt")
                for jc in range(SC):
                    ptr = psB.tile([128, 128], F32, name="ptr", tag="ps512")
                    nc.tensor.transpose(
                        ptr, in_=attn[:, jc * 128:(jc + 1) * 128],
                        identity=ident)
                    nc.vector.tensor_copy(out=attnt[:, jc, :], in_=ptr)
                pctx = psB.tile([64, 128], F32, name="pctx", tag="ps512")
                for jc in range(SC):
                    nc.tensor.matmul(
                        pctx, lhsT=vh_sb[:, jc, lg * 64:(lg + 1) * 64],
                        rhs=attnt[:, jc, :],
                        start=(jc == 0), stop=(jc == SC - 1))
                nc.scalar.copy(out=ctx_sb[hp][hsl, isl], in_=pctx)

    # ---- phase C: partial out-projection ----
    for sc in range(SC):
        po = psB.tile([128, 512], F32, name="po", tag="ps512")
        for hp in range(2):
            nc.tensor.matmul(po, lhsT=ctx_sb[hp][:, sc * 128:(sc + 1) * 128],
                             rhs=wo_sb[:, hp, :], start=(hp == 0), stop=False)
        nc.tensor.matmul(po, lhsT=ones_row[0:1, :128], rhs=bo_sb,
                         start=False, stop=True)
        ot = pop.tile([128, 512], F32, name="ot", tag="ot")
        nc.vector.tensor_copy(out=ot, in_=po)
        nc.sync.dma_start(out=pout[sc * 128:(sc + 1) * 128, :], in_=ot)


_PROGRAM = None


def _get_program():
    global _PROGRAM
    if _PROGRAM is None:
        _PROGRAM = _build_program()
    return _PROGRAM


def host_inputs(q, k, v, mask, Wq, bq_, Wv, bv_, Wo, bo_, gammas):
    """Build the per-core input maps (host-side sharding prep)."""
    def _slopes(n):
        def p2(m):
            start = 2 ** (-(2 ** -(math.log2(m) - 3)))
            return [start * start ** i for i in range(m)]
        if math.log2(n).is_integer():
            return p2(n)
        c = 2 ** math.floor(math.log2(n))
        return p2(c) + _slopes(2 * c)[0::2][: n - c]

    q = np.asarray(q, np.float32)
    k = np.asarray(k, np.float32)
    v = np.asarray(v, np.float32)
    mask2d = np.asarray(mask).reshape(S, S)
    Wq = np.asarray(Wq, np.float32)
    Wv = np.asarray(Wv, np.float32)
    Wo = np.asarray(Wo, np.float32)
    bqf = np.asarray(bq_, np.float32)
    bvf = np.asarray(bv_, np.float32)
    bof = np.asarray(bo_, np.float32)
    gam = np.asarray(gammas, np.float32).reshape(H)

    # rope tables, [dk, s] stacked twice (two heads per partition block)
    inv = 1.0 / (10000.0 ** (np.arange(0, DK, 2, dtype=np.float32) / DK))
    f = np.arange(S, dtype=np.float32)[:, None] * inv[None, :]
    freqs = np.repeat(f, 2, axis=-1)                     # [S, DK]
    cosT = np.ascontiguousarray(np.cos(freqs).T)         # [DK, S]
    sinT = np.ascontiguousarray(np.sin(freqs).T)
    cos2 = np.concatenate([cosT, cosT], 0).astype(np.float32)
    sin2 = np.concatenate([sinT, sinT], 0).astype(np.float32)

    rmat_np = np.zeros((128, 128), np.float32)
    for t in range(64):
        rmat_np[2 * t + 1, 2 * t] = -1.0   # rot[2t]   = -x[2t+1]
        rmat_np[2 * t, 2 * t + 1] = 1.0    # rot[2t+1] =  x[2t]

    import ml_dtypes
    b2_np = np.where(mask2d != 0, 0.0, NEG_BIG).astype(np.float32)
    b2_np = b2_np.reshape(SC, 128, S).astype(ml_dtypes.bfloat16)
    idx = np.arange(S, dtype=np.float32)
    pd_np = np.abs(idx[None, :] - idx[:, None]).astype(np.float32)
    pd_np = np.ascontiguousarray(pd_np.reshape(SC, 128, S)).astype(np.float16)

    slopes = np.asarray(_slopes(H), np.float32)
    sp = np.logaddexp(0.0, gam.astype(np.float64)).astype(np.float32)

    in_maps = []
    for core in range(NCORES):
        b, hg = divmod(core, 2)
        own = slice(hg * HG * DK, (hg + 1) * HG * DK)   # 256 cols
        im = {
            "qT": np.ascontiguousarray(q[b].T).reshape(DC, 128, S),
            "kT": np.ascontiguousarray(k[b].T).reshape(DC, 128, S),
            "vT": np.ascontiguousarray(v[b].T).reshape(DC, 128, S),
            "wqT": np.ascontiguousarray(Wq[own].T).reshape(DC, 128, 256),
            "wvT": np.ascontiguousarray(Wv[own].T).reshape(DC, 128, 256),
            "woT": np.ascontiguousarray(Wo[:, own].T).reshape(2, 128, 512),
            "bq": np.ascontiguousarray(bqf[own]).reshape(1, 256),
            "bv": np.ascontiguousarray(bvf[own]).reshape(1, 256),
            "bo": np.ascontiguousarray(
                bof if core % 2 == 0 else np.zeros_like(bof)).reshape(1, 512),
            "cosq": (cos2 / 8.0).astype(np.float32),
            "sinq": (sin2 / 8.0).astype(np.float32),
            "cosk": cos2,
            "sink": sin2,
            "rmat": rmat_np,
            "b2": b2_np,
            "pdist": pd_np,
            "alibi": np.ascontiguousarray(
                slopes[hg * HG:(hg + 1) * HG, None] * idx[None, :]
            ).astype(np.float16),
            "g2": (-(sp[hg * HG:(hg + 1) * HG] ** 2)).reshape(1, HG)
                  .astype(np.float32),
        }
        in_maps.append(im)
    return in_maps


def kernel(q, k, v, mask, Wq, bq, Wv, bv, Wo, bo, gammas, _trace=False):
    in_maps = host_inputs(q, k, v, mask, Wq, bq, Wv, bv, Wo, bo, gammas)
    nc = _get_program()
    res = run_bass_kernel_spmd(nc, in_maps, list(range(NCORES)),
                               trace=_trace)
    results = res.results

    out = np.zeros((B, S, D), np.float32)
    attn = np.empty((B, H, S, S), np.float32)
    for core in range(NCORES):
        b, hg = divmod(core, 2)
        out[b] += results[core]["pout"]
        attn[b, hg * HG:(hg + 1) * HG] = results[core]["attn_out"]
    kernel._last = res
    return out, attn
